# revision 37
# baseline (speedup 1.0000x reference)
"""Multi-head attention kernel for 8 Trainium2 NeuronCores (v2).

Problem: B=2, S=2048, D=1024, H=16 heads (head_dim 64).
Sharding: data-parallel over batch (2) x tensor-parallel over heads (4 groups
of 4 heads). Core c handles batch c//4, heads [4*(c%4), 4*(c%4)+4).
Each core computes a partial [S, D] output (its heads' contribution through
Wo); the host sums the 4 TP partials per batch.

v2 redesign vs v1:
- Normalization without DRAM round-trips: softmax denominators land at PSUM
  partition 64 via a ones-column in the V stationary; reciprocal_approx_fast
  reads them in place; a rank-1 PE matmul (ones x rec) broadcasts 1/den
  across partitions into PSUM; one DVE multiply fuses normalize + PSUM->SBUF
  eviction of ctx.
- Wo matmuls, Q-projection chunks and the broadcast matmuls are interleaved
  into the attention kt-loops as "fillers" so the PE never drains while the
  scalar engine (exp, the phase-B bottleneck) stays fed.
- Manual PSUM bank map (8 banks): 2x [128,1024] score slots (double-buffered
  exp), 4x [128,512] PV banks (two head-pairs in flight), Wo/denb tiles
  borrow just-freed slots via tag rotation.
- fp16 everywhere (weights incl. Wo, activations, output); fp32 only in PSUM.
"""
import sys

sys.path.insert(0, "/opt/trn_rl_repo")

from collections import deque

import numpy as np

import concourse.bass as bass
import concourse.tile as tile
from concourse import mybir
from concourse import bass_utils

# no fish share in this container; only used when tracing
bass_utils.upload_artifacts = lambda tmpdir: f"local://{tmpdir}"

B, S, D, H = 2, 2048, 1024, 16
HD = 64          # head dim
HL = 4           # heads per core (local)
DL = HL * HD     # local projection dim = 256
N_CORES = 8
SC = 4           # s-chunks of 512 for projections
QC = 4           # q-chunks of 512 for attention
KT = 16          # k-tiles of 128

dt32 = mybir.dt.float32
dtr = mybir.dt.float32r
dtb = mybir.dt.float16

TRACE = False           # set by test.py for profiling runs
LAST_EXEC_NS = None     # stashed by kernel() when TRACE


# ---------------------------------------------------------------- wait split
def _split_waits(nc):
    """Walrus codegen accepts at most one sync wait per instruction on this
    toolchain; move excess waits onto same-engine NoOps inserted before the
    overloaded instruction (engine program order makes this equivalent)."""
    n = 0
    for bb_wrap in nc.main_func.blocks:
        bb = bb_wrap if not hasattr(bb_wrap, "bb") else bb_wrap.bb
        insts = list(bb.instructions)
        out = []
        for ins in insts:
            si = ins.sync_info
            waits = list(si.on_wait) if si is not None else []
            if len(waits) > 1:
                for w in waits[:-1]:
                    nop = mybir.InstNoOp(
                        name=nc.get_next_instruction_name(), ins=[], outs=[]
                    )
                    nop.engine = ins.engine
                    nop.sync_info = mybir.SyncInfo(on_wait=[w], on_update=[])
                    nc.register_instruction(nop)
                    out.append(nop)
                    n += 1
                ins.sync_info = mybir.SyncInfo(
                    on_wait=waits[-1:], on_update=list(si.on_update)
                )
            out.append(ins)
        if len(out) != len(insts):
            bb.instructions = out
    return n


# ---------------------------------------------------------------- program
_PROGRAM = None


def _build_program():
    nc = bass.Bass()
    xq = nc.declare_dram_parameter("xq", [D, S], dtb, isOutput=False)
    xk = nc.declare_dram_parameter("xk", [D, S], dtb, isOutput=False)
    xv = nc.declare_dram_parameter("xv", [D, S], dtb, isOutput=False)
    wq = nc.declare_dram_parameter("wq", [D, DL], dtb, isOutput=False)
    wk = nc.declare_dram_parameter("wk", [D, DL], dtb, isOutput=False)
    wv = nc.declare_dram_parameter("wv", [D, DL], dtb, isOutput=False)
    wo = nc.declare_dram_parameter("wo", [HD, HL, D], dtb, isOutput=False)
    out = nc.declare_dram_parameter("out", [S, D], dtb, isOutput=True)

    with tile.TileContext(nc) as tc:
        with tc.tile_pool(name="const", bufs=1) as const, \
             tc.tile_pool(name="persist", bufs=1) as persist, \
             tc.tile_pool(name="xin", bufs=3) as xin, \
             tc.tile_pool(name="attn", bufs=3) as attn, \
             tc.tile_pool(name="recp", bufs=2) as recp, \
             tc.tile_pool(name="outsb", bufs=3) as outsb, \
             tc.tile_pool(name="dram", bufs=1, space="DRAM") as dram, \
             tc.tile_pool(name="psum", bufs=1, space="PSUM") as psum:

            # ---- resident weights & constants ----
            wqs = const.tile([128, 8, DL], dtb, tag="wq")
            wks = const.tile([128, 8, DL], dtb, tag="wk")
            wvs = const.tile([128, 8, DL], dtb, tag="wv")
            wos = const.tile([HD, HL, D], dtb, tag="wo")

            # ---- persistent activations ----
            qts = persist.tile([128, 2, S], dtb, tag="qts")  # [o%128, o//128, s]
            kts = persist.tile([128, 2, S], dtb, tag="kts")
            # V with a trailing ones column: stationary [v | 1] gives the
            # softmax denominator as PSUM row 64 of the PV accumulation.
            vtsE = persist.tile([128, KT, 2, 66], dtb, tag="vtsE")  # even heads
            vtsO = persist.tile([128, KT, 2, 66], dtb, tag="vtsO")  # odd heads
            ctxE = persist.tile([HD, 8, 512], dtb, tag="ctxE")  # slot = qc*2+hp
            ctxO = persist.tile([HD, 8, 512], dtb, tag="ctxO")

            # weight loads: wk first (needed first) on gpsimd; wq/wo go on
            # the scalar queue AFTER the xk pieces (needed only at ~27us)
            nc.gpsimd.dma_start(out=wks[:], in_=wk[:].rearrange("(ko p) o -> p ko o", p=128))
            nc.gpsimd.dma_start(out=wvs[:], in_=wv[:].rearrange("(ko p) o -> p ko o", p=128))

            nc.vector.memset(vtsE[:], 1.0)
            nc.vector.memset(vtsO[:], 1.0)
            # bias const for the exp-based reciprocal: -12*ln(2)
            nbias = const.tile([128, 1], dt32, tag="nbias")
            nc.vector.memset(nbias[:], -12.0 * 0.6931471805599453)

            # fillers: FIFO of (min_kt, closure) injected into attention
            # loops. min_kt gates how early in a kt-loop a filler may run so
            # a filler whose data isn't ready yet can't block the in-order PE
            # stream. drain_kt pops at most one eligible filler per kt.
            fillers = deque()

            def drain_kt(kt):
                for i, (mk, fn) in enumerate(fillers):
                    if mk <= kt:
                        del fillers[i]
                        fn()
                        return

            def drain_all():
                while fillers:
                    fillers.popleft()[1]()

            sc_tags = ("sc0", "sc1")

            # ---------------- projections ----------------
            def load_x(xdram, xtag, queues):
                pieces = []
                for pc in range(2):
                    d0 = pc * 512
                    t = xin.tile([128, 4, 512], dtb, tag=xtag, name=f"{xtag}p")
                    queues[pc % len(queues)].dma_start(
                        out=t[:],
                        in_=xdram[d0 : d0 + 512, :].rearrange("(ko p) s -> p ko s", p=128)[
                            :, :, 0:512
                        ],
                    )
                    pieces.append(t)
                return pieces

            def proj_qk(c, wtile, pieces, dst, slot):
                p = psum.tile([128, 1024], dt32, tag=sc_tags[slot], name="pqk")
                for ot in range(2):
                    for kc in range(8):
                        nc.tensor.matmul(
                            p[:, ot * 512 : (ot + 1) * 512],
                            wtile[:, kc, ot * 128 : (ot + 1) * 128],
                            pieces[kc // 4][:, kc % 4, :],
                            start=(kc == 0),
                            stop=(kc == 7),
                            skip_group_check=True,
                        )
                for ot in range(2):
                    nc.vector.tensor_copy(
                        dst[:, ot, c * 512 : (c + 1) * 512],
                        p[:, ot * 512 : (ot + 1) * 512],
                    )

            pv_tags = ("pvE", "pvO", "pvE2", "pvO2")

            def proj_v(c, pieces):
                for st in range(4):
                    p = psum.tile([128, 512], dt32, tag=pv_tags[st], name="pv_proj")
                    for kc in range(8):
                        nc.tensor.matmul(
                            p[:, 0:DL],
                            pieces[kc // 4][:, kc % 4, st * 128 : (st + 1) * 128],
                            wvs[:, kc, :],
                            start=(kc == 0),
                            stop=(kc == 7),
                            skip_group_check=True,
                        )
                    idx = c * 4 + st
                    ph = p[:, 0:DL].rearrange("p (h d) -> p h d", h=HL)
                    nc.vector.tensor_copy(vtsE[:, idx, :, 0:HD], ph[:, 0::2, :])
                    nc.vector.tensor_copy(vtsO[:, idx, :, 0:HD], ph[:, 1::2, :])

            # K pieces split p0->sync / p1->scalar so the first chunk lands
            # as fast as the queues allow; V on gpsimd behind the weights
            xk_pieces = [
                load_x(xk[:, c * 512 : (c + 1) * 512], "xk", (nc.sync, nc.scalar))
                for c in range(SC)
            ]
            nc.scalar.dma_start(out=wqs[:], in_=wq[:].rearrange("(ko p) o -> p ko o", p=128))
            nc.scalar.dma_start(out=wos[:], in_=wo[:])
            for c in range(SC):
                proj_qk(c, wks, xk_pieces[c], kts, c % 2)
            xv_pieces = [load_x(xv[:, c * 512 : (c + 1) * 512], "xv", (nc.gpsimd,)) for c in range(SC)]
            for c in range(SC):
                proj_v(c, xv_pieces[c])

            # Q chunk DMAs (sync queue, after xk)
            xq_pieces = [load_x(xq[:, c * 512 : (c + 1) * 512], "xq", (nc.sync,)) for c in range(SC)]
            # Q0 inline (slot s0); Q1-3 become fillers
            proj_qk(0, wqs, xq_pieces[0], qts, 0)

            def push_q_chunk(c):
                # 8 closures x 2 matmuls; the two output halves use separate
                # [128,512] PSUM tiles on banks C/D (freed by the previous
                # norm's multiplies ~kt6, hence min_kt 7). Tiles allocated
                # lazily in their first closure so the tag rotation order
                # matches PE execution order.
                box = {}

                def mk(ot, kc_pair):
                    def go():
                        if ot not in box:
                            box[ot] = psum.tile(
                                [128, 512], dt32, tag=pv_tags[2 + ot], name="pq"
                            )
                        p = box[ot]
                        for kc in kc_pair:
                            nc.tensor.matmul(
                                p[:],
                                wqs[:, kc, ot * 128 : (ot + 1) * 128],
                                xq_pieces[c][kc // 4][:, kc % 4, :],
                                start=(kc == 0),
                                stop=(kc == 7),
                                skip_group_check=True,
                            )
                        if kc_pair[-1] == 7:
                            nc.vector.tensor_copy(
                                qts[:, ot, c * 512 : (c + 1) * 512], p[:]
                            )
                    return go

                for ot in range(2):
                    for kp in ((0, 1), (2, 3), (4, 5), (6, 7)):
                        fillers.append((7, mk(ot, kp)))

            # ---------------- attention + output projection ----------------
            # Deferred norm stages: the previous pair's normalization is
            # emitted INSIDE the next kt-loop (ACT part at kt2, DMA+mults at
            # kt4) so its ln/ln/exp don't delay the next loop's first exps
            # on the ACT engine (that delay stalled the PE's trailing PV).
            pending_norm = []

            def do_norm(qc, hp, pvE_t, pvO_t):
                # Softmax normalization, fully off the PE:
                # - reciprocal on ACT via exp(-ln(den * 2^-12)) = 2^12/den
                #   (custom-DVE recip doesn't compile on this toolchain and
                #   plain DVE reciprocal is ~3.3us; Ln/Exp share one act
                #   table set so there are no table reloads)
                # - partition-broadcast of 1/den via a DRAM round trip on the
                #   (idle in phase B) sync DMA queue
                # - normalize+evict ctx with one DVE scalar_tensor_tensor per
                #   head, folding the 2^-12 back in
                slot = qc * 2 + hp

                def act_part():
                    lnden = recp.tile([128, 2, 512], dt32, tag="rec", name="lnden")
                    rec32 = recp.tile([128, 2, 512], dt32, tag="rec32", name="rec32")
                    nc.scalar.activation(
                        out=lnden[64:65, 0, :], in_=pvE_t[64:65, :],
                        func=mybir.ActivationFunctionType.Ln, scale=2.0 ** -12,
                    )
                    nc.scalar.activation(
                        out=lnden[64:65, 1, :], in_=pvO_t[64:65, :],
                        func=mybir.ActivationFunctionType.Ln, scale=2.0 ** -12,
                    )
                    # exp(-y - 12 ln 2) = 2^-12 * exp(-y) = 1/den directly
                    nc.scalar.activation(
                        out=rec32[64:65, :, :], in_=lnden[64:65, :, :],
                        func=mybir.ActivationFunctionType.Exp, scale=-1.0,
                        bias=nbias[64:65, :],
                    )
                    return rec32

                def rest_part(rec32):
                    recd = dram.tile([2, 512], dt32, tag=f"recd{slot}", name=f"recd{slot}")
                    nc.sync.dma_start(out=recd[:], in_=rec32[64:65, :, :])
                    denbS = recp.tile([HD, 2, 512], dt32, tag="denbS", name="denbS")
                    for hh in range(2):
                        row = recd[hh : hh + 1, :]
                        bc = bass.AP(
                            tensor=row.tensor,
                            offset=row.offset,
                            ap=[[0, HD]] + [list(x) for x in row.ap[1:]],
                        )
                        nc.sync.dma_start(out=denbS[:, hh, :], in_=bc)
                    nc.vector.tensor_mul(
                        ctxE[:, slot, :], pvE_t[0:HD, :], denbS[:, 0, :]
                    )
                    nc.vector.tensor_mul(
                        ctxO[:, slot, :], pvO_t[0:HD, :], denbS[:, 1, :]
                    )

                pending_norm.append((act_part, rest_part))

            def push_wo(qc):
                # 8 closures: one [128,512] out tile each (4 accum matmuls)
                for idx in range(8):
                    t, jc = idx // 2, idx % 2

                    def mk(t, jc, idx):
                        def go():
                            po = psum.tile(
                                [128, 512], dt32, tag=pv_tags[idx % 2], name="po"
                            )
                            for h in range(HL):
                                ctx = ctxE if h % 2 == 0 else ctxO
                                nc.tensor.matmul(
                                    po[:],
                                    ctx[:, qc * 2 + h // 2, t * 128 : (t + 1) * 128],
                                    wos[:, h, jc * 512 : (jc + 1) * 512],
                                    start=(h == 0),
                                    stop=(h == HL - 1),
                                    skip_group_check=True,
                                )
                            ob = outsb.tile([128, 512], dtb, tag="ob", name="ob")
                            nc.vector.tensor_copy(ob[:], po[:])
                            nc.gpsimd.dma_start(
                                out=out[
                                    qc * 512 + t * 128 : qc * 512 + (t + 1) * 128,
                                    jc * 512 : (jc + 1) * 512,
                                ],
                                in_=ob[:],
                            )
                        return go

                    fillers.append((7, mk(t, jc, idx)))

            def attn_loop(qc, hp):
                # Scores double-buffer across kt (psc on sc0/sc1 by parity)
                # so the exp of kt can run while the PE produces scores for
                # kt+1; the PV matmuls trail one kt behind. Each PV pair is
                # split into K=64 halves paired across heads so the two
                # in-flight matmuls occupy disjoint PE row-halves and stream
                # concurrently (like the score pair does).
                pvE_t = psum.tile([128, 512], dt32, tag=pv_tags[2 * hp], name="pvE_t")
                pvO_t = psum.tile([128, 512], dt32, tag=pv_tags[2 * hp + 1], name="pvO_t")
                q0 = qc * 512

                def pv_mms(kt, at):
                    first, last = kt == 0, kt == KT - 1
                    nc.tensor.matmul(
                        pvE_t[0:65, :], vtsE[:, kt, hp, 0:65], at[:, 0:512],
                        start=first, stop=last, skip_group_check=True,
                    )
                    nc.tensor.matmul(
                        pvO_t[0:65, :], vtsO[:, kt, hp, 0:65], at[:, 512:1024],
                        start=first, stop=last, skip_group_check=True,
                    )

                prev_at = None
                norm_state = None
                for kt in range(KT):
                    if kt == 2 and pending_norm:
                        norm_state = pending_norm[0][0]()
                    if kt == 4 and pending_norm:
                        entry = pending_norm.pop(0)
                        entry[1](norm_state)
                    psc = psum.tile([128, 1024], dt32, tag=sc_tags[kt % 2], name="psc")
                    nc.tensor.matmul(
                        psc[:, 0:512],
                        kts[0:64, hp, kt * 128 : (kt + 1) * 128],
                        qts[0:64, hp, q0 : q0 + 512],
                        start=True, stop=True, skip_group_check=True,
                    )
                    nc.tensor.matmul(
                        psc[:, 512:1024],
                        kts[64:128, hp, kt * 128 : (kt + 1) * 128],
                        qts[64:128, hp, q0 : q0 + 512],
                        start=True, stop=True, skip_group_check=True,
                    )
                    at = attn.tile([128, 1024], dtb, tag="at", name="at")
                    nc.scalar.activation(
                        out=at[:],
                        in_=psc[:],
                        func=mybir.ActivationFunctionType.Exp,
                        scale=0.125,
                    )
                    if prev_at is not None:
                        pv_mms(kt - 1, prev_at)
                    prev_at = at
                    if kt >= 2:
                        drain_kt(kt)
                pv_mms(KT - 1, prev_at)
                return pvE_t, pvO_t

            for qc in range(QC):
                # h0 loop drains Q(qc+1) fillers; h1 loop drains Wo(qc-1)
                if qc + 1 < QC:
                    push_q_chunk(qc + 1)
                pvE0, pvO0 = attn_loop(qc, 0)
                do_norm(qc, 0, pvE0, pvO0)
                if qc >= 1:
                    push_wo(qc - 1)
                pvE1, pvO1 = attn_loop(qc, 1)
                do_norm(qc, 1, pvE1, pvO1)

            # tail: flush the last pair's norm, remaining fillers, Wo(q3)
            while pending_norm:
                act_fn, rest_fn = pending_norm.pop(0)
                rest_fn(act_fn())
            drain_all()
            push_wo(QC - 1)
            drain_all()

    _split_waits(nc)
    return nc


def _get_program():
    global _PROGRAM
    if _PROGRAM is None:
        _PROGRAM = _build_program()
    return _PROGRAM


# ---------------------------------------------------------------- host side
def kernel(**inputs):
    global LAST_EXEC_NS
    queries = np.asarray(inputs["queries"], np.float32)
    keys = np.asarray(inputs["keys"], np.float32)
    values = np.asarray(inputs["values"], np.float32)
    Wq = np.asarray(inputs["Wq"], np.float32)
    Wk = np.asarray(inputs["Wk"], np.float32)
    Wv = np.asarray(inputs["Wv"], np.float32)
    Wo = np.asarray(inputs["Wo"], np.float32)

    xT = [np.ascontiguousarray(queries[b].T.astype(np.float16)) for b in range(B)]
    kT = [np.ascontiguousarray(keys[b].T.astype(np.float16)) for b in range(B)]
    vT = [np.ascontiguousarray(values[b].T.astype(np.float16)) for b in range(B)]

    in_maps = []
    for c in range(N_CORES):
        b, g = c // 4, c % 4
        rows = slice(g * DL, (g + 1) * DL)
        wo_p = np.ascontiguousarray(
            Wo[:, rows].T.reshape(HL, HD, D).transpose(1, 0, 2).astype(np.float16)
        )
        in_maps.append({
            "xq": xT[b],
            "xk": kT[b],
            "xv": vT[b],
            "wq": np.ascontiguousarray(Wq[rows, :].T.astype(np.float16)),
            "wk": np.ascontiguousarray(Wk[rows, :].T.astype(np.float16)),
            "wv": np.ascontiguousarray(Wv[rows, :].T.astype(np.float16)),
            "wo": wo_p,
        })

    nc = _get_program()
    res = bass_utils.run_bass_kernel_spmd(
        nc, in_maps, list(range(N_CORES)), trace=TRACE
    )
    if TRACE:
        LAST_EXEC_NS = res.exec_time_ns

    full = np.zeros((B, S, D), np.float32)
    for b in range(B):
        acc = res.results[b * 4 + 0]["out"].astype(np.float32)
        for g in range(1, 4):
            acc = acc + res.results[b * 4 + g]["out"].astype(np.float32)
        full[b] = acc
    return full


# revision 44
# speedup vs baseline: 1.1824x; 1.1824x over previous
"""Multi-head attention kernel for 8 Trainium2 NeuronCores (v2).

Problem: B=2, S=2048, D=1024, H=16 heads (head_dim 64).
Sharding: data-parallel over batch (2) x tensor-parallel over heads (4 groups
of 4 heads). Core c handles batch c//4, heads [4*(c%4), 4*(c%4)+4).
Each core computes a partial [S, D] output (its heads' contribution through
Wo); the host sums the 4 TP partials per batch.

v2 redesign vs v1:
- Normalization without DRAM round-trips: softmax denominators land at PSUM
  partition 64 via a ones-column in the V stationary; reciprocal_approx_fast
  reads them in place; a rank-1 PE matmul (ones x rec) broadcasts 1/den
  across partitions into PSUM; one DVE multiply fuses normalize + PSUM->SBUF
  eviction of ctx.
- Wo matmuls, Q-projection chunks and the broadcast matmuls are interleaved
  into the attention kt-loops as "fillers" so the PE never drains while the
  scalar engine (exp, the phase-B bottleneck) stays fed.
- Manual PSUM bank map (8 banks): 2x [128,1024] score slots (double-buffered
  exp), 4x [128,512] PV banks (two head-pairs in flight), Wo/denb tiles
  borrow just-freed slots via tag rotation.
- fp16 everywhere (weights incl. Wo, activations, output); fp32 only in PSUM.
"""
import sys

sys.path.insert(0, "/opt/trn_rl_repo")

from collections import deque

import numpy as np

import concourse.bass as bass
import concourse.tile as tile
from concourse import mybir
from concourse import bass_utils

# no fish share in this container; only used when tracing
bass_utils.upload_artifacts = lambda tmpdir: f"local://{tmpdir}"

B, S, D, H = 2, 2048, 1024, 16
HD = 64          # head dim
HL = 4           # heads per core (local)
DL = HL * HD     # local projection dim = 256
N_CORES = 8
SC = 4           # s-chunks of 512 for projections
QC = 4           # q-chunks of 512 for attention
KT = 16          # k-tiles of 128

dt32 = mybir.dt.float32
dtr = mybir.dt.float32r
dtb = mybir.dt.float16

TRACE = False           # set by test.py for profiling runs
LAST_EXEC_NS = None     # stashed by kernel() when TRACE


# ---------------------------------------------------------------- wait split
def _split_waits(nc):
    """Walrus codegen accepts at most one sync wait per instruction on this
    toolchain; move excess waits onto same-engine NoOps inserted before the
    overloaded instruction (engine program order makes this equivalent)."""
    n = 0
    for bb_wrap in nc.main_func.blocks:
        bb = bb_wrap if not hasattr(bb_wrap, "bb") else bb_wrap.bb
        insts = list(bb.instructions)
        out = []
        for ins in insts:
            si = ins.sync_info
            waits = list(si.on_wait) if si is not None else []
            if len(waits) > 1:
                for w in waits[:-1]:
                    nop = mybir.InstNoOp(
                        name=nc.get_next_instruction_name(), ins=[], outs=[]
                    )
                    nop.engine = ins.engine
                    nop.sync_info = mybir.SyncInfo(on_wait=[w], on_update=[])
                    nc.register_instruction(nop)
                    out.append(nop)
                    n += 1
                ins.sync_info = mybir.SyncInfo(
                    on_wait=waits[-1:], on_update=list(si.on_update)
                )
            out.append(ins)
        if len(out) != len(insts):
            bb.instructions = out
    return n


# ---------------------------------------------------------------- program
_PROGRAM = None


def _build_program():
    nc = bass.Bass()
    xq = nc.declare_dram_parameter("xq", [D, S], dtb, isOutput=False)
    xk = nc.declare_dram_parameter("xk", [D, S], dtb, isOutput=False)
    xv = nc.declare_dram_parameter("xv", [D, S], dtb, isOutput=False)
    wq = nc.declare_dram_parameter("wq", [D, DL], dtb, isOutput=False)
    wk = nc.declare_dram_parameter("wk", [D, DL], dtb, isOutput=False)
    wv = nc.declare_dram_parameter("wv", [D, DL], dtb, isOutput=False)
    wo = nc.declare_dram_parameter("wo", [HD, HL, D], dtb, isOutput=False)
    out = nc.declare_dram_parameter("out", [S, D], dtb, isOutput=True)

    with tile.TileContext(nc) as tc:
        with tc.tile_pool(name="const", bufs=1) as const, \
             tc.tile_pool(name="persist", bufs=1) as persist, \
             tc.tile_pool(name="xin", bufs=3) as xin, \
             tc.tile_pool(name="attn", bufs=3) as attn, \
             tc.tile_pool(name="recp", bufs=2) as recp, \
             tc.tile_pool(name="outsb", bufs=3) as outsb, \
             tc.tile_pool(name="dram", bufs=1, space="DRAM") as dram, \
             tc.tile_pool(name="psum", bufs=1, space="PSUM") as psum:

            # ---- resident weights & constants ----
            wqs = const.tile([128, 8, DL], dtb, tag="wq")
            wks = const.tile([128, 8, DL], dtb, tag="wk")
            wvs = const.tile([128, 8, DL], dtb, tag="wv")
            wos = const.tile([HD, HL, D], dtb, tag="wo")

            # ---- persistent activations ----
            qts = persist.tile([128, 2, S], dtb, tag="qts")  # [o%128, o//128, s]
            kts = persist.tile([128, 2, S], dtb, tag="kts")
            # V with a trailing ones column: stationary [v | 1] gives the
            # softmax denominator as PSUM row 64 of the PV accumulation.
            vtsE = persist.tile([128, KT, 2, 66], dtb, tag="vtsE")  # even heads
            vtsO = persist.tile([128, KT, 2, 66], dtb, tag="vtsO")  # odd heads
            ctxE = persist.tile([HD, 8, 512], dtb, tag="ctxE")  # slot = qc*2+hp
            ctxO = persist.tile([HD, 8, 512], dtb, tag="ctxO")

            # weight loads: wk first (needed first) on gpsimd; wq/wo go on
            # the scalar queue AFTER the xk pieces (needed only at ~27us)
            nc.gpsimd.dma_start(out=wks[:], in_=wk[:].rearrange("(ko p) o -> p ko o", p=128))
            nc.gpsimd.dma_start(out=wvs[:], in_=wv[:].rearrange("(ko p) o -> p ko o", p=128))

            nc.vector.memset(vtsE[:], 1.0)
            nc.vector.memset(vtsO[:], 1.0)
            # bias const for the exp-based reciprocal: -12*ln(2)
            nbias = const.tile([128, 1], dt32, tag="nbias")
            nc.vector.memset(nbias[:], -12.0 * 0.6931471805599453)
            # f16 ones for the tail's PE-broadcast of 1/den
            ones16 = const.tile([128, HD], dtb, tag="ones16")
            nc.vector.memset(ones16[:], 1.0)

            # fillers: FIFO of (min_kt, closure) injected into attention
            # loops. min_kt gates how early in a kt-loop a filler may run so
            # a filler whose data isn't ready yet can't block the in-order PE
            # stream. drain_kt pops at most one eligible filler per kt.
            fillers = deque()

            def drain_kt(kt):
                for i, (mk, fn) in enumerate(fillers):
                    if mk <= kt:
                        del fillers[i]
                        fn()
                        return

            def drain_all():
                while fillers:
                    fillers.popleft()[1]()

            sc_tags = ("sc0", "sc1")

            # ---------------- projections ----------------
            def load_x(xdram, xtag, queues):
                pieces = []
                for pc in range(2):
                    d0 = pc * 512
                    t = xin.tile([128, 4, 512], dtb, tag=xtag, name=f"{xtag}p")
                    queues[pc % len(queues)].dma_start(
                        out=t[:],
                        in_=xdram[d0 : d0 + 512, :].rearrange("(ko p) s -> p ko s", p=128)[
                            :, :, 0:512
                        ],
                    )
                    pieces.append(t)
                return pieces

            def proj_qk(c, wtile, pieces, dst, slot):
                p = psum.tile([128, 1024], dt32, tag=sc_tags[slot], name="pqk")
                for ot in range(2):
                    for kc in range(8):
                        nc.tensor.matmul(
                            p[:, ot * 512 : (ot + 1) * 512],
                            wtile[:, kc, ot * 128 : (ot + 1) * 128],
                            pieces[kc // 4][:, kc % 4, :],
                            start=(kc == 0),
                            stop=(kc == 7),
                            skip_group_check=True,
                        )
                for ot in range(2):
                    nc.vector.tensor_copy(
                        dst[:, ot, c * 512 : (c + 1) * 512],
                        p[:, ot * 512 : (ot + 1) * 512],
                    )

            pv_tags = ("pvE", "pvO", "pvE2", "pvO2")

            def proj_v(c, pieces):
                for st in range(4):
                    p = psum.tile([128, 512], dt32, tag=pv_tags[st], name="pv_proj")
                    for kc in range(8):
                        nc.tensor.matmul(
                            p[:, 0:DL],
                            pieces[kc // 4][:, kc % 4, st * 128 : (st + 1) * 128],
                            wvs[:, kc, :],
                            start=(kc == 0),
                            stop=(kc == 7),
                            skip_group_check=True,
                        )
                    idx = c * 4 + st
                    ph = p[:, 0:DL].rearrange("p (h d) -> p h d", h=HL)
                    nc.vector.tensor_copy(vtsE[:, idx, :, 0:HD], ph[:, 0::2, :])
                    nc.vector.tensor_copy(vtsO[:, idx, :, 0:HD], ph[:, 1::2, :])

            # K pieces split p0->sync / p1->scalar so the first chunk lands
            # as fast as the queues allow; V on gpsimd behind the weights
            xk_pieces = [
                load_x(xk[:, c * 512 : (c + 1) * 512], "xk", (nc.sync, nc.scalar))
                for c in range(SC)
            ]
            nc.scalar.dma_start(out=wqs[:], in_=wq[:].rearrange("(ko p) o -> p ko o", p=128))
            nc.scalar.dma_start(out=wos[:], in_=wo[:])
            for c in range(SC):
                proj_qk(c, wks, xk_pieces[c], kts, c % 2)
            xv_pieces = [load_x(xv[:, c * 512 : (c + 1) * 512], "xv", (nc.gpsimd,)) for c in range(SC)]
            for c in range(SC):
                proj_v(c, xv_pieces[c])

            # Q chunk DMAs (sync queue, after xk)
            xq_pieces = [load_x(xq[:, c * 512 : (c + 1) * 512], "xq", (nc.sync,)) for c in range(SC)]
            # Q0 inline (slot s0); Q1-3 become fillers
            proj_qk(0, wqs, xq_pieces[0], qts, 0)

            def push_q_chunk(c):
                # 8 closures x 2 matmuls; the two output halves use separate
                # [128,512] PSUM tiles on banks C/D (freed by the previous
                # norm's multiplies ~kt6, hence min_kt 7). Tiles allocated
                # lazily in their first closure so the tag rotation order
                # matches PE execution order.
                box = {}

                def mk(ot, kc_pair):
                    def go():
                        if ot not in box:
                            box[ot] = psum.tile(
                                [128, 512], dt32, tag=pv_tags[2 + ot], name="pq"
                            )
                        p = box[ot]
                        for kc in kc_pair:
                            nc.tensor.matmul(
                                p[:],
                                wqs[:, kc, ot * 128 : (ot + 1) * 128],
                                xq_pieces[c][kc // 4][:, kc % 4, :],
                                start=(kc == 0),
                                stop=(kc == 7),
                                skip_group_check=True,
                            )
                        if kc_pair[-1] == 7:
                            nc.vector.tensor_copy(
                                qts[:, ot, c * 512 : (c + 1) * 512], p[:]
                            )
                    return go

                for ot in range(2):
                    for kp in ((0, 1), (2, 3), (4, 5), (6, 7)):
                        fillers.append((7, mk(ot, kp)))

            # ---------------- attention + output projection ----------------
            # Deferred norm stages: the previous pair's normalization is
            # emitted INSIDE the next kt-loop (ACT part at kt2, DMA+mults at
            # kt4) so its ln/ln/exp don't delay the next loop's first exps
            # on the ACT engine (that delay stalled the PE's trailing PV).
            pending_norm = []

            def do_norm(qc, hp, pvE_t, pvO_t, tail=False):
                # Softmax normalization, fully off the PE:
                # - reciprocal on ACT via exp(-ln(den * 2^-12)) = 2^12/den
                #   (custom-DVE recip doesn't compile on this toolchain and
                #   plain DVE reciprocal is ~3.3us; Ln/Exp share one act
                #   table set so there are no table reloads)
                # - partition-broadcast of 1/den via a DRAM round trip on the
                #   (idle in phase B) sync DMA queue
                # - normalize+evict ctx with one DVE scalar_tensor_tensor per
                #   head, folding the 2^-12 back in
                slot = qc * 2 + hp

                def act_part():
                    lnden = recp.tile([128, 2, 512], dt32, tag="rec", name="lnden")
                    rec32 = recp.tile([128, 2, 512], dt32, tag="rec32", name="rec32")
                    nc.scalar.activation(
                        out=lnden[64:65, 0, :], in_=pvE_t[64:65, :],
                        func=mybir.ActivationFunctionType.Ln, scale=2.0 ** -12,
                    )
                    nc.scalar.activation(
                        out=lnden[64:65, 1, :], in_=pvO_t[64:65, :],
                        func=mybir.ActivationFunctionType.Ln, scale=2.0 ** -12,
                    )
                    # exp(-y - 12 ln 2) = 2^-12 * exp(-y) = 1/den directly
                    nc.scalar.activation(
                        out=rec32[64:65, :, :], in_=lnden[64:65, :, :],
                        func=mybir.ActivationFunctionType.Exp, scale=-1.0,
                        bias=nbias[64:65, :],
                    )
                    return rec32

                def rest_part(rec32):
                    recd = dram.tile([2, 512], dt32, tag=f"recd{slot}", name=f"recd{slot}")
                    nc.sync.dma_start(out=recd[:], in_=rec32[64:65, :, :])
                    denbS = recp.tile([HD, 2, 512], dt32, tag="denbS", name="denbS")
                    for hh in range(2):
                        row = recd[hh : hh + 1, :]
                        bc = bass.AP(
                            tensor=row.tensor,
                            offset=row.offset,
                            ap=[[0, HD]] + [list(x) for x in row.ap[1:]],
                        )
                        nc.sync.dma_start(out=denbS[:, hh, :], in_=bc)
                    nc.vector.tensor_mul(
                        ctxE[:, slot, :], pvE_t[0:HD, :], denbS[:, 0, :]
                    )
                    nc.vector.tensor_mul(
                        ctxO[:, slot, :], pvO_t[0:HD, :], denbS[:, 1, :]
                    )

                def tail_part():
                    # tail: banks A/B are free and nothing else runs, so a
                    # rank-1 PE broadcast beats the DRAM round trip's DMA
                    # completion latencies
                    lnden = recp.tile([128, 2, 512], dt32, tag="rec", name="lnden")
                    rec16 = recp.tile([128, 2, 512], dtb, tag="rec16t", name="rec16t")
                    nc.scalar.activation(
                        out=lnden[64:65, 0, :], in_=pvE_t[64:65, :],
                        func=mybir.ActivationFunctionType.Ln, scale=2.0 ** -12,
                    )
                    nc.scalar.activation(
                        out=lnden[64:65, 1, :], in_=pvO_t[64:65, :],
                        func=mybir.ActivationFunctionType.Ln, scale=2.0 ** -12,
                    )
                    nc.scalar.activation(
                        out=rec16[64:65, :, :], in_=lnden[64:65, :, :],
                        func=mybir.ActivationFunctionType.Exp, scale=-1.0,
                        bias=nbias[64:65, :],
                    )
                    denbE = psum.tile([128, 512], dt32, tag=pv_tags[0], name="denbE")
                    denbO = psum.tile([128, 512], dt32, tag=pv_tags[1], name="denbO")
                    nc.tensor.matmul(
                        denbE[0:HD, :], ones16[64:65, :], rec16[64:65, 0, :],
                        start=True, stop=True, skip_group_check=True,
                    )
                    nc.tensor.matmul(
                        denbO[0:HD, :], ones16[64:65, :], rec16[64:65, 1, :],
                        start=True, stop=True, skip_group_check=True,
                    )
                    denbS = recp.tile([HD, 2, 512], dt32, tag="denbS", name="denbSt")
                    nc.vector.tensor_copy(denbS[:, 0, :], denbE[0:HD, :])
                    nc.vector.tensor_copy(denbS[:, 1, :], denbO[0:HD, :])
                    nc.vector.tensor_mul(
                        ctxE[:, slot, :], pvE_t[0:HD, :], denbS[:, 0, :]
                    )
                    nc.vector.tensor_mul(
                        ctxO[:, slot, :], pvO_t[0:HD, :], denbS[:, 1, :]
                    )

                if tail:
                    tail_part()
                else:
                    rest_part(act_part())

            def push_wo(qc):
                # 8 closures: one [128,512] out tile each (4 accum matmuls)
                for idx in range(8):
                    t, jc = idx // 2, idx % 2

                    def mk(t, jc, idx):
                        def go():
                            po = psum.tile(
                                [128, 512], dt32, tag=pv_tags[idx % 2], name="po"
                            )
                            for h in range(HL):
                                ctx = ctxE if h % 2 == 0 else ctxO
                                nc.tensor.matmul(
                                    po[:],
                                    ctx[:, qc * 2 + h // 2, t * 128 : (t + 1) * 128],
                                    wos[:, h, jc * 512 : (jc + 1) * 512],
                                    start=(h == 0),
                                    stop=(h == HL - 1),
                                    skip_group_check=True,
                                )
                            ob = outsb.tile([128, 512], dtb, tag="ob", name="ob")
                            nc.vector.tensor_copy(ob[:], po[:])
                            nc.gpsimd.dma_start(
                                out=out[
                                    qc * 512 + t * 128 : qc * 512 + (t + 1) * 128,
                                    jc * 512 : (jc + 1) * 512,
                                ],
                                in_=ob[:],
                            )
                        return go

                    fillers.append((7, mk(t, jc, idx)))

            def attn_loop(qc, hp):
                # Scores double-buffer across kt (psc on sc0/sc1 by parity)
                # so the exp of kt can run while the PE produces scores for
                # kt+1; the PV matmuls trail one kt behind. Each PV pair is
                # split into K=64 halves paired across heads so the two
                # in-flight matmuls occupy disjoint PE row-halves and stream
                # concurrently (like the score pair does).
                pvE_t = psum.tile([128, 512], dt32, tag=pv_tags[2 * hp], name="pvE_t")
                pvO_t = psum.tile([128, 512], dt32, tag=pv_tags[2 * hp + 1], name="pvO_t")
                q0 = qc * 512

                def pv_mms(kt, at):
                    first, last = kt == 0, kt == KT - 1
                    nc.tensor.matmul(
                        pvE_t[0:65, :], vtsE[:, kt, hp, 0:65], at[:, 0:512],
                        start=first, stop=last, skip_group_check=True,
                    )
                    nc.tensor.matmul(
                        pvO_t[0:65, :], vtsO[:, kt, hp, 0:65], at[:, 512:1024],
                        start=first, stop=last, skip_group_check=True,
                    )

                prev_at = None
                for kt in range(KT):
                    psc = psum.tile([128, 1024], dt32, tag=sc_tags[kt % 2], name="psc")
                    nc.tensor.matmul(
                        psc[:, 0:512],
                        kts[0:64, hp, kt * 128 : (kt + 1) * 128],
                        qts[0:64, hp, q0 : q0 + 512],
                        start=True, stop=True, skip_group_check=True,
                    )
                    nc.tensor.matmul(
                        psc[:, 512:1024],
                        kts[64:128, hp, kt * 128 : (kt + 1) * 128],
                        qts[64:128, hp, q0 : q0 + 512],
                        start=True, stop=True, skip_group_check=True,
                    )
                    at = attn.tile([128, 1024], dtb, tag="at", name="at")
                    nc.scalar.activation(
                        out=at[:],
                        in_=psc[:],
                        func=mybir.ActivationFunctionType.Exp,
                        scale=0.125,
                    )
                    if prev_at is not None:
                        pv_mms(kt - 1, prev_at)
                    prev_at = at
                    if kt >= 2:
                        drain_kt(kt)
                pv_mms(KT - 1, prev_at)
                return pvE_t, pvO_t

            for qc in range(QC):
                # h0 loop drains Q(qc+1) fillers; h1 loop drains Wo(qc-1)
                if qc + 1 < QC:
                    push_q_chunk(qc + 1)
                pvE0, pvO0 = attn_loop(qc, 0)
                do_norm(qc, 0, pvE0, pvO0)
                if qc >= 1:
                    push_wo(qc - 1)
                pvE1, pvO1 = attn_loop(qc, 1)
                do_norm(qc, 1, pvE1, pvO1, tail=(qc == QC - 1))

            # tail: finish remaining fillers, then Wo(q3)
            drain_all()
            push_wo(QC - 1)
            drain_all()

    _split_waits(nc)
    return nc


def _get_program():
    global _PROGRAM
    if _PROGRAM is None:
        _PROGRAM = _build_program()
    return _PROGRAM


# ---------------------------------------------------------------- host side
def kernel(**inputs):
    global LAST_EXEC_NS
    queries = np.asarray(inputs["queries"], np.float32)
    keys = np.asarray(inputs["keys"], np.float32)
    values = np.asarray(inputs["values"], np.float32)
    Wq = np.asarray(inputs["Wq"], np.float32)
    Wk = np.asarray(inputs["Wk"], np.float32)
    Wv = np.asarray(inputs["Wv"], np.float32)
    Wo = np.asarray(inputs["Wo"], np.float32)

    xT = [np.ascontiguousarray(queries[b].T.astype(np.float16)) for b in range(B)]
    kT = [np.ascontiguousarray(keys[b].T.astype(np.float16)) for b in range(B)]
    vT = [np.ascontiguousarray(values[b].T.astype(np.float16)) for b in range(B)]

    in_maps = []
    for c in range(N_CORES):
        b, g = c // 4, c % 4
        rows = slice(g * DL, (g + 1) * DL)
        wo_p = np.ascontiguousarray(
            Wo[:, rows].T.reshape(HL, HD, D).transpose(1, 0, 2).astype(np.float16)
        )
        in_maps.append({
            "xq": xT[b],
            "xk": kT[b],
            "xv": vT[b],
            "wq": np.ascontiguousarray(Wq[rows, :].T.astype(np.float16)),
            "wk": np.ascontiguousarray(Wk[rows, :].T.astype(np.float16)),
            "wv": np.ascontiguousarray(Wv[rows, :].T.astype(np.float16)),
            "wo": wo_p,
        })

    nc = _get_program()
    res = bass_utils.run_bass_kernel_spmd(
        nc, in_maps, list(range(N_CORES)), trace=TRACE
    )
    if TRACE:
        LAST_EXEC_NS = res.exec_time_ns

    full = np.zeros((B, S, D), np.float32)
    for b in range(B):
        acc = res.results[b * 4 + 0]["out"].astype(np.float32)
        for g in range(1, 4):
            acc = acc + res.results[b * 4 + g]["out"].astype(np.float32)
        full[b] = acc
    return full
